# revision 1
# baseline (speedup 1.0000x reference)
"""Trainium2 Bass kernel for single-head causal attention (B=4, T=4096, C=2048, HS=128).

Sharding: 2 cores per batch element (8 cores, B=4), interleaved 512-row
q-chunks: role A (even cores) owns global chunks [0,2,4,6], role B (odd)
[1,3,5,7]. Each core projects Q^T/K^T/V^T for its own 2048 rows, AllGathers
K^T/V^T within its pair via DRAM staging, and runs causal attention over
its 4 q-slots with static per-slot extents of 2(s+1) 512-chunks.

Attention is split own-chunks-first / partner-chunks-second so the
AllGather overlaps the Q projections + the own half of attention. Scores
for 3 k-tiles at a time land in a 3-bank PSUM tile and are exponentiated
by one batched ACT instruction; AV partials and column-packed
(tile_position) denominator partials accumulate per group in a 2-bank
PSUM tile and are folded into per-slot SBUF accumulators by the vector
engine, with the AV matmuls lagging the score matmuls by two groups so
the exp stream and the PE stream overlap. Softmax normalization uses a
broadcast matmul (ones/32 stationary) + reciprocal_approx_fast. The
output is y^T per slot, transposed on the host. Partner addressing is
role-independent: both parities of the gathered buffer are blended with
per-core 0/1 selectors, so all 8 cores run one SPMD graph.
"""

import math
from collections import deque

import numpy as np
import ml_dtypes

import concourse.bacc as bacc
import concourse.tile as tile
from concourse import mybir
from concourse.bass_utils import run_bass_kernel_spmd

B, T, C, HS = 4, 4096, 2048, 128
NCORES = 8
TOWN = 2048              # sequence rows owned per core
NCT = C // 128           # 16 contraction tiles
QTILES_A = [0, 2, 4, 6]
QTILES_B = [1, 3, 5, 7]

BF16 = ml_dtypes.bfloat16


def build_graph(with_collective=True, sbuf_transpose=True):
    nc = bacc.Bacc(
        "TRN2", target_bir_lowering=False, debug=False, num_devices=NCORES
    )
    bf = mybir.dt.bfloat16
    f32 = mybir.dt.float32
    EXP = mybir.ActivationFunctionType.Exp

    xt_d = nc.dram_tensor("xt", [128, NCT, TOWN], bf, kind="ExternalInput")
    w3_d = nc.dram_tensor("w3", [128, 3, NCT, HS], bf, kind="ExternalInput")
    mo_d = nc.dram_tensor("mo", [128, 2048], bf, kind="ExternalInput")
    ps0_d = nc.dram_tensor("ps0", [128, 1], f32, kind="ExternalInput")
    ps1_d = nc.dram_tensor("ps1", [128, 1], f32, kind="ExternalInput")
    psz_d = nc.dram_tensor("psz", [128, 1], f32, kind="ExternalInput")
    # y^T per slot, normalized; host transposes to [512, HS]
    out_d = nc.dram_tensor("out", [4, 128, 512], f32, kind="ExternalOutput")

    with tile.TileContext(nc) as tc:
        with (
            tc.tile_pool(name="big", bufs=1) as big,
            tc.tile_pool(name="dram", bufs=1, space="DRAM") as dram,
        ):
            # ---- loads: x c-tiles round-robin in consumption order;
            # w3 pieces and small tensors interleaved to balance queues ----
            w3 = big.tile([128, 3, NCT, HS], bf, tag="w3")
            xt = big.tile([128, NCT, TOWN], bf, tag="xt")

            nc.scalar.dma_start(w3[:, 1:3, 0:4, :], w3_d[:, 1:3, 0:4, :])
            qs = [nc.sync, nc.scalar, nc.gpsimd]
            for c in range(NCT):
                qs[c % 3].dma_start(xt[:, c : c + 1, :], xt_d[:, c : c + 1, :])
                if c == 4:
                    nc.scalar.dma_start(
                        w3[:, 1:3, 4:10, :], w3_d[:, 1:3, 4:10, :]
                    )
                elif c == 10:
                    nc.scalar.dma_start(
                        w3[:, 1:3, 10:16, :], w3_d[:, 1:3, 10:16, :]
                    )
            ps0 = big.tile([128, 1], f32, tag="ps0")
            nc.gpsimd.dma_start(ps0[:], ps0_d[:])
            ps1 = big.tile([128, 1], f32, tag="ps1")
            nc.gpsimd.dma_start(ps1[:], ps1_d[:])
            psz = big.tile([128, 1], f32, tag="psz")
            nc.gpsimd.dma_start(psz[:], psz_d[:])
            nc.gpsimd.dma_start(w3[:, 0:1, :, :], w3_d[:, 0:1, :, :])
            mo = big.tile([128, 2048], bf, tag="mo")
            nc.gpsimd.dma_start(mo[:], mo_d[:])

            # ---- constants ----
            wut = big.tile([128, 512], bf, tag="wut")
            nc.vector.memset(wut[:], 0.0)
            on32 = big.tile([128, 32], bf, tag="on32")
            nc.vector.memset(on32[:], 1.0)
            scb = big.tile([128, 128], f32, tag="scb")
            nc.vector.memset(scb[:], 1.0 / 32.0)

            # ---- persistent SBUF tensors ----
            ktq = big.tile([128, TOWN], bf, tag="ktq")  # own K^T (slot order)
            vtq = big.tile([128, TOWN], bf, tag="vtq")  # own V^T
            v3o = big.tile([128, 16, HS], bf, tag="v3o")  # own V (k-major)
            qts = [
                big.tile([128, 512], bf, tag=f"qt{s}", name=f"qt{s}")
                for s in range(4)
            ]
            ktp = [
                big.tile([128, TOWN], bf, tag=f"ktp{r}", name=f"ktp{r}")
                for r in range(2)
            ]
            v3p = [
                big.tile([128, 16, HS], bf, tag=f"v3p{r}", name=f"v3p{r}")
                for r in range(2)
            ]
            ktpar = big.tile([128, TOWN], bf, tag="ktpar")
            v3par = big.tile([128, 16, HS], bf, tag="v3par")
            accs = [
                big.tile([128, 1024], f32, tag=f"acc{s}", name=f"acc{s}")
                for s in range(4)
            ]

            kvb = dram.tile([256, TOWN], bf, tag="kvb")
            kvg = dram.tile([512, TOWN], bf, tag="kvg")
            vst = dram.tile([128, TOWN], bf, tag="vst")

            # ---- projections: K,V then Q for own rows ----
            with tc.tile_pool(name="pjps", bufs=8, space="PSUM") as pjps:
                # PE warm-up while the first input DMAs land
                wup = pjps.tile([128, 512], f32, tag="pj", name="wup")
                for _ in range(28):
                    nc.tensor.matmul(
                        wup[:], wut[:, 0:128], wut[:], start=True, stop=True
                    )
                ps8 = [
                    pjps.tile([128, 512], f32, tag="pj", name=f"pa{i}")
                    for i in range(8)
                ]
                for c in range(NCT):
                    for wi in range(2):  # 0 = K, 1 = V
                        for t4 in range(4):
                            nc.tensor.matmul(
                                ps8[wi * 4 + t4][:],
                                w3[:, wi + 1, c, :],
                                xt[:, c, t4 * 512 : (t4 + 1) * 512],
                                start=(c == 0),
                                stop=(c == NCT - 1),
                            )
                for wi, dest in [(0, ktq), (1, vtq)]:
                    for t4 in range(4):
                        nc.vector.tensor_copy(
                            dest[:, t4 * 512 : (t4 + 1) * 512],
                            ps8[wi * 4 + t4][:],
                        )
                # staging writes on two queues in parallel
                nc.sync.dma_start(kvb[0:128, :], ktq[:])
                nc.scalar.dma_start(kvb[128:256, :], vtq[:])
                if sbuf_transpose:
                    nc.sync.dma_start_transpose(v3o[:], vtq[:])
                else:
                    nc.scalar.dma_start(vst[:], vtq[:])
                    nc.scalar.dma_start_transpose(v3o[:], vst[:])
                if with_collective:
                    nc.gpsimd.collective_compute(
                        "AllGather",
                        mybir.AluOpType.bypass,
                        replica_groups=[[0, 1], [2, 3], [4, 5], [6, 7]],
                        ins=[kvb.opt()],
                        outs=[kvg.opt()],
                    )
                else:  # timeline-model stub: same data volume, no comms
                    nc.scalar.dma_start(kvg[0:256, :], kvb[:])
                    nc.scalar.dma_start(kvg[256:512, :], kvb[:])
                # partner halves (block on the collective; tail of queues)
                nc.gpsimd.dma_start(ktp[0][:], kvg[0:128, :])
                nc.gpsimd.dma_start(ktp[1][:], kvg[256:384, :])
                nc.sync.dma_start_transpose(v3p[0][:], kvg[128:256, :])
                nc.sync.dma_start_transpose(v3p[1][:], kvg[384:512, :])

                # Q projections for all 4 slots (reuses the pj PSUM ring)
                pq = [
                    pjps.tile([128, 512], f32, tag="pj", name=f"pq{s}")
                    for s in range(4)
                ]
                for c in range(NCT):
                    for s in range(4):
                        nc.tensor.matmul(
                            pq[s][:],
                            w3[:, 0, c, :],
                            xt[:, c, s * 512 : (s + 1) * 512],
                            start=(c == 0),
                            stop=(c == NCT - 1),
                        )
                for s in range(4):
                    nc.vector.tensor_copy(qts[s][:], pq[s][:])

            # role-independent partner buffers: blend the two parities with
            # per-core 0/1 selectors (ps0 = partner-is-parity-0). Emitted
            # between the own and partner phases so the vector queue is not
            # head-of-line blocked on the collective during the own phase.
            def emit_blends():
                nc.vector.tensor_scalar_mul(ktp[0][:], ktp[0][:], ps0[:])
                nc.vector.tensor_scalar_mul(ktp[1][:], ktp[1][:], ps1[:])
                nc.vector.tensor_add(ktpar[:], ktp[0][:], ktp[1][:])
                nc.vector.tensor_scalar_mul(v3p[0][:], v3p[0][:], ps0[:])
                nc.vector.tensor_scalar_mul(v3p[1][:], v3p[1][:], ps1[:])
                nc.vector.tensor_add(v3par[:], v3p[0][:], v3p[1][:])

            # ---- attention ----
            with (
                tc.tile_pool(name="srng", bufs=2, space="PSUM") as srng,
                tc.tile_pool(name="pps", bufs=1, space="PSUM") as pps,
                tc.tile_pool(name="pp", bufs=4) as pp,
                tc.tile_pool(name="ep", bufs=2) as ep,
            ):
                def groups_of(ntiles):
                    out, i = [], 0
                    while i < ntiles:
                        n = min(3, ntiles - i)
                        out.append((i, n))
                        i += n
                    return out

                # scores + batched exp (+ causal mask) for one group
                def emit_scores(s, g0, n, own, mask_lo):
                    kt = ktq if own else ktpar
                    S = srng.tile([128, 1536], f32, tag="r", name="sg")
                    for i in range(n):
                        t = g0 + i
                        nc.tensor.matmul(
                            S[:, i * 512 : (i + 1) * 512],
                            kt[:, t * 128 : (t + 1) * 128],
                            qts[s][:],
                            start=True,
                            stop=True,
                        )
                    p = pp.tile([128, 1536], bf, tag="p", name="pg")
                    nc.scalar.activation(p[:, 0 : n * 512], S[:, 0 : n * 512], EXP)
                    # own diag chunk: structural mask; partner last chunk:
                    # per-core scalar (zero for role A, one for role B)
                    lo = max(g0, mask_lo) - g0
                    if g0 + n > mask_lo:
                        sl = p[:, lo * 512 : n * 512]
                        if own:
                            j = g0 + lo - mask_lo
                            nc.vector.tensor_mul(
                                sl, sl, mo[:, j * 512 : (j + n - lo) * 512]
                            )
                        else:
                            nc.vector.tensor_scalar_mul(sl, sl, psz[:])
                    return p

                # AV + column-packed denominator partials for one group
                def emit_av(s, g0, n, own, p, first):
                    v3 = v3o if own else v3par
                    P = pps.tile([128, 1024], f32, tag="pv", name="pv")
                    for i in range(n):
                        t = g0 + i
                        nc.tensor.matmul(
                            P[:, 0:512],
                            v3[:, t, :],
                            p[:, i * 512 : (i + 1) * 512],
                            start=(i == 0),
                            stop=(i == n - 1),
                            skip_group_check=True,
                        )
                    for i in range(4):
                        mv = (
                            p[:, i * 512 : (i + 1) * 512]
                            if i < n
                            else wut[:, 0:512]
                        )
                        nc.tensor.matmul(
                            P[32 * i : 32 * i + 32, 512:1024],
                            on32[:],
                            mv,
                            start=True,
                            stop=True,
                            skip_group_check=True,
                            tile_position=(0, 32 * i),
                        )
                    if first:
                        nc.vector.tensor_copy(accs[s][:], P[:, 0:1024])
                    else:
                        nc.vector.tensor_add(accs[s][:], accs[s][:], P[:, 0:1024])

                # software pipeline, AV lagging scores by two groups
                pending = deque()

                def emit_phase(s, own):
                    ntiles = 4 * (s + 1)
                    mask_lo = ntiles - 4
                    first = own
                    for g0, n in groups_of(ntiles):
                        p = emit_scores(s, g0, n, own, mask_lo)
                        if len(pending) >= 2:
                            emit_av(*pending.popleft())
                        pending.append((s, g0, n, own, p, first and g0 == 0))

                def drain():
                    while pending:
                        emit_av(*pending.popleft())

                for s in range(4):
                    emit_phase(s, True)
                emit_blends()
                for s in (3, 2, 1, 0):
                    emit_phase(s, False)
                    drain()  # acc[s] complete before its epilogue
                    FD = srng.tile([128, 1536], f32, tag="r", name=f"fd{s}")
                    nc.tensor.matmul(
                        FD[:, 0:512],
                        scb[:],
                        accs[s][:, 512:1024],
                        start=True,
                        stop=True,
                    )
                    fdc = ep.tile([128, 512], f32, tag="fdc", name=f"fdc{s}")
                    nc.vector.tensor_copy(fdc[:], FD[:, 0:512])
                    rb = ep.tile([128, 512], f32, tag="rb", name=f"rb{s}")
                    nc.vector.reciprocal_approx_fast(rb[:], fdc[:])
                    ot = ep.tile([128, 512], f32, tag="ot", name=f"ot{s}")
                    nc.vector.tensor_mul(ot[:], accs[s][:, 0:512], rb[:])
                    nc.gpsimd.dma_start(out_d[s], ot[:])

    nc.compile()
    return nc


def _role_qtiles(h):
    return QTILES_A if h == 0 else QTILES_B


def _diag_mask():
    """[128, 4*512] bf16: tile j of the diagonal 512-chunk, k<=q."""
    m = np.zeros((128, 4, 512), np.float32)
    k = np.arange(128)[:, None]
    q = np.arange(512)[None, :]
    for j in range(4):
        m[:, j, :] = (128 * j + k <= q).astype(np.float32)
    return np.ascontiguousarray(m.reshape(128, 2048)).astype(BF16)


def make_in_maps(x, Wq, Wk, Wv):
    """Host-side sharding + layout prep. x [B,T,C] f32, W* [C,HS] f32."""
    wq_s = np.asarray(Wq, np.float32) / math.sqrt(HS)
    w3 = np.stack(
        [wq_s, np.asarray(Wk, np.float32), np.asarray(Wv, np.float32)]
    )
    w3_arr = np.ascontiguousarray(
        w3.reshape(3, NCT, 128, HS).transpose(2, 0, 1, 3)
    ).astype(BF16)
    mo = _diag_mask()

    in_maps = []
    for core in range(NCORES):
        b, h = core // 2, core % 2
        qtiles = _role_qtiles(h)
        rows = np.concatenate(
            [np.arange(g * 512, (g + 1) * 512) for g in qtiles]
        )
        xr = np.asarray(x[b])[rows]  # [2048 rows, C] f32
        xT = np.ascontiguousarray(xr.T).astype(BF16)  # [C, 2048]
        xt_arr = np.ascontiguousarray(
            xT.reshape(NCT, 128, TOWN).transpose(1, 0, 2)
        )  # [128, NCT, 2048]
        in_maps.append(
            {
                "xt": xt_arr,
                "w3": w3_arr,
                "mo": mo,
                # partner parity selectors: partner parity = 1-h
                "ps0": np.full((128, 1), float(h), np.float32),
                "ps1": np.full((128, 1), float(1 - h), np.float32),
                # partner last chunk: fully masked for role A, visible for B
                "psz": np.full((128, 1), float(h), np.float32),
            }
        )
    return in_maps


def assemble_out(results):
    """results: list of 8 dicts with 'out' [4,128,512] -> y [B,T,HS] f32."""
    y = np.zeros((B, T, HS), np.float32)
    for core in range(NCORES):
        b, h = core // 2, core % 2
        qtiles = _role_qtiles(h)
        o = np.asarray(results[core]["out"])  # [4, 128, 512] = y^T per slot
        for s in range(4):
            g = qtiles[s]
            y[b, g * 512 : (g + 1) * 512] = o[s].T
    return y


_NC_CACHE = None


def _get_graph():
    global _NC_CACHE
    if _NC_CACHE is None:
        _NC_CACHE = build_graph()
    return _NC_CACHE


def kernel(x, Wq, Wk, Wv):
    import time

    nc = _get_graph()
    in_maps = make_in_maps(x, Wq, Wk, Wv)
    try:
        res = run_bass_kernel_spmd(nc, in_maps, list(range(NCORES)))
    except Exception:
        time.sleep(15)  # transient device/mesh hiccup: one retry
        res = run_bass_kernel_spmd(nc, in_maps, list(range(NCORES)))
    return assemble_out(res.results)



# revision 6
# speedup vs baseline: 1.0102x; 1.0102x over previous
"""Trainium2 Bass kernel for single-head causal attention (B=4, T=4096, C=2048, HS=128).

Sharding: 2 cores per batch element (8 cores, B=4). No collective: each
core loads the FULL batch x (16MB bf16) and projects all K/V itself.
Role-independent SPMD via pair-swapped storage order: core with role h
stores chunk position p = global chunk p^h, so its own 4 q-chunks sit at
fixed positions (0,2,4,6) and the causal k-extent of slot s is always the
position-prefix of length 2s+2 chunks, with the odd position 2s+1 either
fully masked (role A) or fully valid (role B) via a per-core 0/1 scalar.

Pipeline: K^T for all 8 chunks accumulates across the 16 c-tiles in all
8 PSUM banks while xt streams in (DMA-bound phase). Post-DMA, per slot:
Q projection and two V-chunk projections (PSUM bank borrowed from the
score pool, V transposed to k-major via DMA transpose), then scores in
3-tile PSUM groups double-buffered, one batched exp per group on the
scalar engine, causal masks on the vector engine, and AV + column-packed
denominator partials accumulating directly in two persistent PSUM banks
across the whole slot (has_written semantics; no vector folds). Epilogue:
1/32-broadcast matmul + reciprocal_approx_fast + multiply, DMA out y^T;
host transposes.
"""

import math

import numpy as np
import ml_dtypes

import concourse.bacc as bacc
import concourse.tile as tile
from concourse import mybir
from concourse.bass_utils import run_bass_kernel_spmd

B, T, C, HS = 4, 4096, 2048, 128
NCORES = 8
NCT = C // 128           # 16 contraction tiles
NPOS = 8                 # 512-row chunk positions per batch

BF16 = ml_dtypes.bfloat16


def _groups_of(ntiles):
    """Split ntiles into groups of 3 (avoiding trailing 1-tile groups)."""
    out, i = [], 0
    while i < ntiles:
        rem = ntiles - i
        n = 3 if (rem >= 5 or rem == 3) else (2 if rem >= 2 else 1)
        if rem == 4:
            n = 2
        out.append((i, n))
        i += n
    return out


def build_graph():
    nc = bacc.Bacc(
        "TRN2", target_bir_lowering=False, debug=False, num_devices=NCORES
    )
    bf = mybir.dt.bfloat16
    f32 = mybir.dt.float32
    EXP = mybir.ActivationFunctionType.Exp

    xt_d = nc.dram_tensor("xt", [128, NCT, T], bf, kind="ExternalInput")
    w3_d = nc.dram_tensor("w3", [128, 3, NCT, HS], bf, kind="ExternalInput")
    mo_d = nc.dram_tensor("mo", [128, 2048], bf, kind="ExternalInput")
    psz_d = nc.dram_tensor("psz", [128, 1], f32, kind="ExternalInput")
    # y^T per slot, normalized; host transposes to [512, HS]
    out_d = nc.dram_tensor("out", [4, 128, 512], f32, kind="ExternalOutput")

    with tile.TileContext(nc) as tc:
        with tc.tile_pool(name="big", bufs=1) as big:
            # ---- persistent SBUF tensors ----
            w3 = big.tile([128, 3, NCT, HS], bf, tag="w3")
            xt = big.tile([128, NCT, T], bf, tag="xt")
            mo = big.tile([128, 2048], bf, tag="mo")
            psz = big.tile([128, 1], f32, tag="psz")
            on32 = big.tile([128, 32], bf, tag="on32")
            scb = big.tile([128, 128], f32, tag="scb")
            ktq = big.tile([128, T], bf, tag="ktq")        # K^T all positions
            vtq = big.tile([128, T], bf, tag="vtq")        # V^T all positions
            v3 = big.tile([128, 32, HS], bf, tag="v3")     # V k-major
            qts = [
                big.tile([128, 512], bf, tag=f"qt{s}", name=f"qt{s}")
                for s in range(4)
            ]

            # ---- input DMAs: weights first, then xt round-robin ----
            nc.sync.dma_start(w3[:, :, 0:4, :], w3_d[:, :, 0:4, :])
            qs = [nc.sync, nc.scalar, nc.gpsimd]
            for c in range(NCT):
                qs[c % 3].dma_start(xt[:, c : c + 1, :], xt_d[:, c : c + 1, :])
                if c == 3:
                    nc.scalar.dma_start(w3[:, :, 4:10, :], w3_d[:, :, 4:10, :])
                elif c == 7:
                    nc.gpsimd.dma_start(
                        w3[:, :, 10:16, :], w3_d[:, :, 10:16, :]
                    )
            nc.gpsimd.dma_start(mo[:], mo_d[:])
            nc.gpsimd.dma_start(psz[:], psz_d[:])

            # ---- constants ----
            nc.vector.memset(on32[:], 1.0)
            nc.vector.memset(scb[:], 1.0 / 32.0)

            # ---- phase A: K^T for all 8 positions, accumulating in all
            # 8 PSUM banks while xt streams in (c-outer) ----
            with tc.tile_pool(name="kps", bufs=1, space="PSUM") as kps:
                kb = [
                    kps.tile([128, 512], f32, tag=f"kb{p}", name=f"kb{p}")
                    for p in range(NPOS)
                ]
                for c in range(NCT):
                    for p in range(NPOS):
                        nc.tensor.matmul(
                            kb[p][:],
                            w3[:, 1, c, :],
                            xt[:, c, p * 512 : (p + 1) * 512],
                            start=(c == 0),
                            stop=(c == NCT - 1),
                        )
                for p in range(NPOS):
                    nc.vector.tensor_copy(
                        ktq[:, p * 512 : (p + 1) * 512], kb[p][:]
                    )

            # ---- phase B: attention slots with V/Q passes interleaved ----
            with (
                tc.tile_pool(name="srng", bufs=2, space="PSUM") as srng,
                tc.tile_pool(name="acc", bufs=1, space="PSUM") as accp,
                tc.tile_pool(name="pp", bufs=4) as pp,
                tc.tile_pool(name="ep", bufs=2) as ep,
            ):
                av = accp.tile([128, 512], f32, tag="av", name="av")
                den = accp.tile([128, 512], f32, tag="den", name="den")

                def proj_pass(dst_bf, wi, pos):
                    """Project one 512-chunk (K-layout) into dst_bf via a
                    borrowed score-pool bank."""
                    t512 = srng.tile(
                        [128, 1536], f32, tag="r", name=f"pj{wi}_{pos}"
                    )
                    for c in range(NCT):
                        nc.tensor.matmul(
                            t512[:, 0:512],
                            w3[:, wi, c, :],
                            xt[:, c, pos * 512 : (pos + 1) * 512],
                            start=(c == 0),
                            stop=(c == NCT - 1),
                        )
                    nc.vector.tensor_copy(dst_bf[:], t512[:, 0:512])

                for s in range(4):
                    # Q for this slot; V for the two new positions
                    proj_pass(qts[s][:], 0, 2 * s)
                    for pos in (2 * s, 2 * s + 1):
                        proj_pass(vtq[:, pos * 512 : (pos + 1) * 512], 2, pos)
                        nc.sync.dma_start_transpose(
                            v3[:, 4 * pos : 4 * pos + 4, :],
                            vtq[:, pos * 512 : (pos + 1) * 512],
                        )

                    E = 8 * s + 8  # k-tiles in this slot's extent
                    # zero the den bank; den matmuls then accumulate (or
                    # overwrite-with-equal-effect) regardless of stale
                    # has_written state from the previous slot
                    nc.vector.memset(den[:], 0.0)
                    for g0, n in _groups_of(E):
                        S = srng.tile([128, 1536], f32, tag="r", name="sg")
                        for i in range(n):
                            t = g0 + i
                            nc.tensor.matmul(
                                S[:, i * 512 : (i + 1) * 512],
                                ktq[:, t * 128 : (t + 1) * 128],
                                qts[s][:],
                                start=True,
                                stop=True,
                            )
                        p = pp.tile([128, 1536], bf, tag="p", name="pg")
                        nc.scalar.activation(
                            p[:, 0 : n * 512], S[:, 0 : n * 512], EXP
                        )
                        # masks: diag chunk at tiles 8s..8s+3 (structural),
                        # odd chunk at tiles 8s+4..8s+7 (per-core selector)
                        for i in range(n):
                            t = g0 + i
                            sl = p[:, i * 512 : (i + 1) * 512]
                            if 8 * s <= t < 8 * s + 4:
                                j = t - 8 * s
                                nc.vector.tensor_mul(
                                    sl, sl, mo[:, j * 512 : (j + 1) * 512]
                                )
                            elif t >= 8 * s + 4:
                                nc.vector.tensor_scalar_mul(sl, sl, psz[:])
                        # AV + den accumulate across the slot in PSUM
                        for i in range(n):
                            t = g0 + i
                            nc.tensor.matmul(
                                av[:],
                                v3[:, t, :],
                                p[:, i * 512 : (i + 1) * 512],
                                start=(t == 0),
                                stop=(t == E - 1),
                                skip_group_check=True,
                            )
                            cg = t % 4
                            nc.tensor.matmul(
                                den[32 * cg : 32 * cg + 32, :],
                                on32[:],
                                p[:, i * 512 : (i + 1) * 512],
                                start=False,
                                stop=(t >= E - 4),
                                skip_group_check=True,
                                tile_position=(0, 32 * cg),
                            )

                    # epilogue: broadcast den, reciprocal, normalize, store
                    dsb = ep.tile([128, 512], f32, tag="dsb", name=f"dsb{s}")
                    nc.vector.tensor_copy(dsb[:], den[:])
                    FD = srng.tile([128, 1536], f32, tag="r", name=f"fd{s}")
                    nc.tensor.matmul(
                        FD[:, 0:512], scb[:], dsb[:], start=True, stop=True
                    )
                    fdc = ep.tile([128, 512], f32, tag="fdc", name=f"fdc{s}")
                    nc.vector.tensor_copy(fdc[:], FD[:, 0:512])
                    rb = ep.tile([128, 512], f32, tag="rb", name=f"rb{s}")
                    nc.vector.reciprocal_approx_fast(rb[:], fdc[:])
                    ot = ep.tile([128, 512], f32, tag="ot", name=f"ot{s}")
                    nc.vector.tensor_mul(ot[:], av[:], rb[:])
                    nc.gpsimd.dma_start(out_d[s], ot[:])

    nc.compile()
    return nc


def _diag_mask():
    """[128, 4*512] bf16: tile j of the diagonal 512-chunk, k<=q."""
    m = np.zeros((128, 4, 512), np.float32)
    k = np.arange(128)[:, None]
    q = np.arange(512)[None, :]
    for j in range(4):
        m[:, j, :] = (128 * j + k <= q).astype(np.float32)
    return np.ascontiguousarray(m.reshape(128, 2048)).astype(BF16)


def make_in_maps(x, Wq, Wk, Wv):
    """Host-side layout prep. x [B,T,C] f32, W* [C,HS] f32."""
    wq_s = np.asarray(Wq, np.float32) / math.sqrt(HS)
    w3 = np.stack(
        [wq_s, np.asarray(Wk, np.float32), np.asarray(Wv, np.float32)]
    )
    w3_arr = np.ascontiguousarray(
        w3.reshape(3, NCT, 128, HS).transpose(2, 0, 1, 3)
    ).astype(BF16)
    mo = _diag_mask()

    in_maps = []
    for core in range(NCORES):
        b, h = core // 2, core % 2
        rows = np.concatenate(
            [np.arange((p ^ h) * 512, ((p ^ h) + 1) * 512) for p in range(NPOS)]
        )
        xr = np.asarray(x[b])[rows]  # [4096 rows in sigma order, C] f32
        xT = np.ascontiguousarray(xr.T).astype(BF16)  # [C, 4096]
        xt_arr = np.ascontiguousarray(
            xT.reshape(NCT, 128, T).transpose(1, 0, 2)
        )  # [128, NCT, 4096]
        in_maps.append(
            {
                "xt": xt_arr,
                "w3": w3_arr,
                "mo": mo,
                # odd positions 2s+1: masked for role A, valid for role B
                "psz": np.full((128, 1), float(h), np.float32),
            }
        )
    return in_maps


def assemble_out(results):
    """results: list of 8 dicts with 'out' [4,128,512] -> y [B,T,HS] f32."""
    y = np.zeros((B, T, HS), np.float32)
    for core in range(NCORES):
        b, h = core // 2, core % 2
        o = np.asarray(results[core]["out"])  # [4, 128, 512] = y^T per slot
        for s in range(4):
            g = (2 * s) ^ h
            y[b, g * 512 : (g + 1) * 512] = o[s].T
    return y


_NC_CACHE = None


def _get_graph():
    global _NC_CACHE
    if _NC_CACHE is None:
        _NC_CACHE = build_graph()
    return _NC_CACHE


def kernel(x, Wq, Wk, Wv):
    import time

    nc = _get_graph()
    in_maps = make_in_maps(x, Wq, Wk, Wv)
    try:
        res = run_bass_kernel_spmd(nc, in_maps, list(range(NCORES)))
    except Exception:
        time.sleep(15)  # transient device/mesh hiccup: one retry
        res = run_bass_kernel_spmd(nc, in_maps, list(range(NCORES)))
    return assemble_out(res.results)


# revision 8
# speedup vs baseline: 1.1286x; 1.1171x over previous
"""Trainium2 Bass kernel for single-head causal attention (B=4, T=4096, C=2048, HS=128).

Sharding: 2 cores per batch element (8 cores, B=4). No collective: each
core loads the FULL batch x (16MB bf16) and projects all K/V itself.
Role-independent SPMD via pair-swapped storage order: core with role h
stores chunk position p = global chunk p^h, so its own 4 q-chunks sit at
fixed positions (0,2,4,6) and the causal k-extent of slot s is always the
position-prefix of length 2s+2 chunks, with the odd position 2s+1 either
fully masked (role A) or fully valid (role B) via a per-core 0/1 scalar.

Pipeline: K^T for all 8 chunks accumulates across the 16 c-tiles in all
8 PSUM banks while xt streams in (DMA-bound phase). Post-DMA, per slot:
Q projection and two V-chunk projections (PSUM bank borrowed from the
score pool, V transposed to k-major via DMA transpose), then scores in
3-tile PSUM groups double-buffered, one batched exp per group on the
scalar engine, causal masks on the vector engine, and AV + column-packed
denominator partials accumulating directly in two persistent PSUM banks
across the whole slot (has_written semantics; no vector folds). Epilogue:
1/32-broadcast matmul + reciprocal_approx_fast + multiply, DMA out y^T;
host transposes.
"""

import math

import numpy as np
import ml_dtypes

import concourse.bacc as bacc
import concourse.tile as tile
from concourse import mybir
from concourse.bass_utils import run_bass_kernel_spmd

B, T, C, HS = 4, 4096, 2048, 128
NCORES = 8
NCT = C // 128           # 16 contraction tiles
NPOS = 8                 # 512-row chunk positions per batch

BF16 = ml_dtypes.bfloat16


def _groups_of(ntiles):
    """Split ntiles into groups of 3 (avoiding trailing 1-tile groups)."""
    out, i = [], 0
    while i < ntiles:
        rem = ntiles - i
        n = 3 if (rem >= 5 or rem == 3) else (2 if rem >= 2 else 1)
        if rem == 4:
            n = 2
        out.append((i, n))
        i += n
    return out


def build_graph():
    nc = bacc.Bacc(
        "TRN2", target_bir_lowering=False, debug=False, num_devices=NCORES
    )
    bf = mybir.dt.bfloat16
    f32 = mybir.dt.float32
    EXP = mybir.ActivationFunctionType.Exp

    xt_d = nc.dram_tensor("xt", [128, NCT, T], bf, kind="ExternalInput")
    w3_d = nc.dram_tensor("w3", [128, 3, NCT, HS], bf, kind="ExternalInput")
    mo_d = nc.dram_tensor("mo", [128, 2048], bf, kind="ExternalInput")
    psz_d = nc.dram_tensor("psz", [128, 1], f32, kind="ExternalInput")
    # y^T per slot, normalized; host transposes to [512, HS]
    out_d = nc.dram_tensor("out", [4, 128, 512], f32, kind="ExternalOutput")

    with tile.TileContext(nc) as tc:
        with tc.tile_pool(name="big", bufs=1) as big:
            # ---- persistent SBUF tensors ----
            w3 = big.tile([128, 3, NCT, HS], bf, tag="w3")
            xt = big.tile([128, NCT, T], bf, tag="xt")
            mo = big.tile([128, 2048], bf, tag="mo")
            psz = big.tile([128, 1], f32, tag="psz")
            on32 = big.tile([128, 32], bf, tag="on32")
            scb = big.tile([128, 128], bf, tag="scb")
            ktq = big.tile([128, T], bf, tag="ktq")        # K^T all positions
            vtq = big.tile([128, T], bf, tag="vtq")        # V^T all positions
            v3 = big.tile([128, 32, HS], bf, tag="v3")     # V k-major
            qts = [
                big.tile([128, 512], bf, tag=f"qt{s}", name=f"qt{s}")
                for s in range(4)
            ]

            # ---- input DMAs: weights first, then xt round-robin ----
            nc.sync.dma_start(w3[:, :, 0:4, :], w3_d[:, :, 0:4, :])
            qs = [nc.sync, nc.scalar, nc.gpsimd]
            for c in range(NCT):
                qs[c % 3].dma_start(xt[:, c : c + 1, :], xt_d[:, c : c + 1, :])
                if c == 3:
                    nc.scalar.dma_start(w3[:, :, 4:10, :], w3_d[:, :, 4:10, :])
                elif c == 7:
                    nc.gpsimd.dma_start(
                        w3[:, :, 10:16, :], w3_d[:, :, 10:16, :]
                    )
            nc.gpsimd.dma_start(mo[:], mo_d[:])
            nc.gpsimd.dma_start(psz[:], psz_d[:])

            # ---- constants ----
            nc.vector.memset(on32[:], 1.0)
            nc.vector.memset(scb[:], 1.0 / 32.0)

            # ---- phase A: K^T for all 8 positions, accumulating in all
            # 8 PSUM banks while xt streams in (c-outer) ----
            with tc.tile_pool(name="kps", bufs=1, space="PSUM") as kps:
                kb = [
                    kps.tile([128, 512], f32, tag=f"kb{p}", name=f"kb{p}")
                    for p in range(NPOS)
                ]
                for c in range(NCT):
                    for p in range(NPOS):
                        nc.tensor.matmul(
                            kb[p][:],
                            w3[:, 1, c, :],
                            xt[:, c, p * 512 : (p + 1) * 512],
                            start=(c == 0),
                            stop=(c == NCT - 1),
                        )
                for p in range(NPOS):
                    nc.vector.tensor_copy(
                        ktq[:, p * 512 : (p + 1) * 512], kb[p][:]
                    )

            # ---- phase B: attention slots with V/Q passes interleaved
            # as PE-bubble fillers ----
            with (
                tc.tile_pool(name="srng", bufs=2, space="PSUM") as srng,
                tc.tile_pool(name="acc", bufs=1, space="PSUM") as accp,
                tc.tile_pool(name="fbp", bufs=2, space="PSUM") as fbp,
                tc.tile_pool(name="pp", bufs=4) as pp,
                tc.tile_pool(name="ep", bufs=2) as ep,
            ):
                av = accp.tile([128, 512], f32, tag="av", name="av")
                den = accp.tile([128, 512], f32, tag="den", name="den")

                def proj_ops(dst_bf, wi, pos, transpose_to=None):
                    """Yield one op per call: 16 accumulating MMs into a
                    filler PSUM bank, then the copy-out (and V transpose)."""
                    fb = fbp.tile([128, 512], f32, tag="fb", name=f"fb{wi}_{pos}")
                    for c in range(NCT):
                        def mm(c=c):
                            nc.tensor.matmul(
                                fb[:],
                                w3[:, wi, c, :],
                                xt[:, c, pos * 512 : (pos + 1) * 512],
                                start=(c == 0),
                                stop=(c == NCT - 1),
                            )
                        yield mm
                    def fin():
                        nc.vector.tensor_copy(dst_bf[:], fb[:])
                        if transpose_to is not None:
                            nc.sync.dma_start_transpose(transpose_to, dst_bf[:])
                    yield fin

                def preamble(s):
                    yield from proj_ops(qts[s][:], 0, 2 * s)
                    for pos in (2 * s, 2 * s + 1):
                        yield from proj_ops(
                            vtq[:, pos * 512 : (pos + 1) * 512], 2, pos,
                            transpose_to=v3[:, 4 * pos : 4 * pos + 4, :],
                        )

                def drain(gen):
                    if gen is not None:
                        for op in gen:
                            op()

                def emit_fillers(gen, k):
                    if gen is None:
                        return
                    for _ in range(k):
                        op = next(gen, None)
                        if op is None:
                            return
                        op()

                # slot 0 and 1 preambles run before attention starts
                drain(preamble(0))
                drain(preamble(1))

                for s in range(4):
                    E = 8 * s + 8  # k-tiles in this slot's extent
                    G = E // 2     # 2-tile score groups
                    filler = preamble(s + 1) if s + 1 < 4 else None
                    nc.vector.memset(den[:], 0.0)

                    def emit_scores(g):
                        S = srng.tile([128, 1024], f32, tag="r", name="sg")
                        for i in range(2):
                            t = 2 * g + i
                            nc.tensor.matmul(
                                S[:, i * 512 : (i + 1) * 512],
                                ktq[:, t * 128 : (t + 1) * 128],
                                qts[s][:],
                                start=True,
                                stop=True,
                            )
                        p = pp.tile([128, 1024], bf, tag="p", name="pg")
                        nc.scalar.activation(p[:], S[:], EXP)
                        for i in range(2):
                            t = 2 * g + i
                            sl = p[:, i * 512 : (i + 1) * 512]
                            if 8 * s <= t < 8 * s + 4:
                                j = t - 8 * s
                                nc.vector.tensor_mul(
                                    sl, sl, mo[:, j * 512 : (j + 1) * 512]
                                )
                            elif t >= 8 * s + 4:
                                nc.vector.tensor_scalar_mul(sl, sl, psz[:])
                        return p

                    def emit_av(g, p):
                        for i in range(2):
                            t = 2 * g + i
                            nc.tensor.matmul(
                                av[:],
                                v3[:, t, :],
                                p[:, i * 512 : (i + 1) * 512],
                                start=(t == 0),
                                stop=(t == E - 1),
                                skip_group_check=True,
                            )
                        for i in range(2):
                            t = 2 * g + i
                            cg = t % 4
                            nc.tensor.matmul(
                                den[32 * cg : 32 * cg + 32, :],
                                on32[:],
                                p[:, i * 512 : (i + 1) * 512],
                                start=False,
                                stop=(t >= E - 4),
                                skip_group_check=True,
                                tile_position=(0, 32 * cg),
                            )

                    # software pipeline: scores lead AV by one group
                    prev = emit_scores(0)
                    for g in range(1, G):
                        cur = emit_scores(g)
                        emit_av(g - 1, prev)
                        prev = cur
                        emit_fillers(filler, 6)
                    emit_av(G - 1, prev)
                    drain(filler)

                    # epilogue: broadcast den, reciprocal, normalize, store
                    dsb = ep.tile([128, 512], bf, tag="dsb", name=f"dsb{s}")
                    nc.vector.tensor_copy(dsb[:], den[:])
                    FD = srng.tile([128, 1024], f32, tag="r", name=f"fd{s}")
                    nc.tensor.matmul(
                        FD[:, 0:512], scb[:], dsb[:], start=True, stop=True
                    )
                    fdc = ep.tile([128, 512], f32, tag="fdc", name=f"fdc{s}")
                    nc.vector.tensor_copy(fdc[:], FD[:, 0:512])
                    rb = ep.tile([128, 512], f32, tag="rb", name=f"rb{s}")
                    nc.vector.reciprocal_approx_fast(rb[:], fdc[:])
                    ot = ep.tile([128, 512], f32, tag="ot", name=f"ot{s}")
                    nc.vector.tensor_mul(ot[:], av[:], rb[:])
                    nc.gpsimd.dma_start(out_d[s], ot[:])

    nc.compile()
    return nc


def _diag_mask():
    """[128, 4*512] bf16: tile j of the diagonal 512-chunk, k<=q."""
    m = np.zeros((128, 4, 512), np.float32)
    k = np.arange(128)[:, None]
    q = np.arange(512)[None, :]
    for j in range(4):
        m[:, j, :] = (128 * j + k <= q).astype(np.float32)
    return np.ascontiguousarray(m.reshape(128, 2048)).astype(BF16)


def make_in_maps(x, Wq, Wk, Wv):
    """Host-side layout prep. x [B,T,C] f32, W* [C,HS] f32."""
    wq_s = np.asarray(Wq, np.float32) / math.sqrt(HS)
    w3 = np.stack(
        [wq_s, np.asarray(Wk, np.float32), np.asarray(Wv, np.float32)]
    )
    w3_arr = np.ascontiguousarray(
        w3.reshape(3, NCT, 128, HS).transpose(2, 0, 1, 3)
    ).astype(BF16)
    mo = _diag_mask()

    in_maps = []
    for core in range(NCORES):
        b, h = core // 2, core % 2
        rows = np.concatenate(
            [np.arange((p ^ h) * 512, ((p ^ h) + 1) * 512) for p in range(NPOS)]
        )
        xr = np.asarray(x[b])[rows]  # [4096 rows in sigma order, C] f32
        xT = np.ascontiguousarray(xr.T).astype(BF16)  # [C, 4096]
        xt_arr = np.ascontiguousarray(
            xT.reshape(NCT, 128, T).transpose(1, 0, 2)
        )  # [128, NCT, 4096]
        in_maps.append(
            {
                "xt": xt_arr,
                "w3": w3_arr,
                "mo": mo,
                # odd positions 2s+1: masked for role A, valid for role B
                "psz": np.full((128, 1), float(h), np.float32),
            }
        )
    return in_maps


def assemble_out(results):
    """results: list of 8 dicts with 'out' [4,128,512] -> y [B,T,HS] f32."""
    y = np.zeros((B, T, HS), np.float32)
    for core in range(NCORES):
        b, h = core // 2, core % 2
        o = np.asarray(results[core]["out"])  # [4, 128, 512] = y^T per slot
        for s in range(4):
            g = (2 * s) ^ h
            y[b, g * 512 : (g + 1) * 512] = o[s].T
    return y


_NC_CACHE = None


def _get_graph():
    global _NC_CACHE
    if _NC_CACHE is None:
        _NC_CACHE = build_graph()
    return _NC_CACHE


def kernel(x, Wq, Wk, Wv):
    import time

    nc = _get_graph()
    in_maps = make_in_maps(x, Wq, Wk, Wv)
    try:
        res = run_bass_kernel_spmd(nc, in_maps, list(range(NCORES)))
    except Exception:
        time.sleep(15)  # transient device/mesh hiccup: one retry
        res = run_bass_kernel_spmd(nc, in_maps, list(range(NCORES)))
    return assemble_out(res.results)
